# revision 27
# baseline (speedup 1.0000x reference)
"""RBF similarity: out[b, n] = exp(-gamma * ||inputs[b] - sample_matrix[n]||^2).

Strategy (8 trn2 NeuronCores, data-parallel over query rows):
  - Shard B=8192 query rows into 8 shards of 1024; replicate sample_matrix.
  - GEMM trick: -gamma*||x-s||^2 = 2g*x.s - g*||x||^2 - g*||s||^2.
  - Device computes psum = x_bf16.T @ s_bf16 over K=256, plus a K=32
    "tail" k-tile whose rows carry -0.5*||s||^2 (hi/lo bf16 split keeps
    the norm at ~fp32 precision). The tail weights live in 4
    partition-replicated 32-row groups so the 4 PSUM banks' K=32 matmuls
    run concurrently in disjoint row groups of the PE array
    (tile_position packing). Only rows 0/1 of each group are nonzero, so
    the tail s-rows are 4 tiny DMAs and the tail x-weights are memsets.
  - One ScalarE activation per 4-bank PSUM half evicts
    exp(2g*psum - g*||x||^2 + ln(65535)) as uint16 fixed point (the
    per-partition bias carries the per-row terms); the host decodes the
    uint16 transport back to fp32 in the gather (quantization error
    <= 0.5/65535, far below the bf16 matmul noise).
  - Raw bass with manual semaphores: the walrus build here allows at
    most one sync-wait per instruction, which Tile's scheduler exceeds.
  - Inputs arrive as 2 packed [128, 5120] bf16 stripes (x cols | s cols
    per k-group), each loaded by two DMAs on different HWDGE rings
    (sync + scalar) so per-engine descriptor latency overlaps; the PE
    starts right after stripe 1 and the first two halves interleave
    with the rest of the load.
"""

import numpy as np
import ml_dtypes

import concourse.bass as bass
import concourse.mybir as mybir
from concourse.bass import ts
from concourse.bass_utils import run_bass_kernel_spmd

GAMMA = 0.001
B, D, N = 8192, 256, 4096
NCORES = 8
B_LOC = B // NCORES          # 1024 query rows per core
M_TILES = B_LOC // 128       # 8 PSUM-partition tiles
KTAIL = 32                   # tail k-tile (rows 0/1: -0.5*||s||^2 hi/lo)
NB = 512                     # matmul free dim = one PSUM bank (fp32)
HALF = 2048                  # 4 banks per PSUM half
HALVES = 2 * M_TILES         # 16 half-iterations
PACK = B_LOC + N             # 5120: packed stripe width (x cols | s cols)
SPLIT = 2560                 # ring-split point of a stripe

BF16 = mybir.dt.bfloat16
F32 = mybir.dt.float32
U16 = mybir.dt.uint16
OUT_SCALE = 65535.0  # device writes round(out * 65535) as uint16; host rescales


def _build() -> bass.Bass:
    nc = bass.Bass(name="rbf_similarity", trn_type="TRN2")
    in1 = nc.dram_tensor("in1", [128, PACK], BF16, kind="ExternalInput")
    in2 = nc.dram_tensor("in2", [128, PACK], BF16, kind="ExternalInput")
    in3 = nc.dram_tensor("in3", [128, PACK], BF16, kind="ExternalInput")
    xsq = nc.dram_tensor("xsq", [128, M_TILES], F32, kind="ExternalInput")
    out = nc.dram_tensor("out", [B_LOC, N], U16, kind="ExternalOutput")

    with (
        nc.sbuf_tensor([128, PACK], BF16) as t1,
        nc.sbuf_tensor([128, PACK], BF16) as t2,
        nc.sbuf_tensor([128, PACK], BF16) as t3,
        nc.sbuf_tensor([128, M_TILES], F32) as xq,
        nc.sbuf_tensor([128, 1], F32) as scratch,
        nc.sbuf_tensor([128, N], U16) as ot0,
        nc.sbuf_tensor([128, N], U16) as ot1,
        nc.sbuf_tensor([128, N], U16) as ot2,
        nc.sbuf_tensor([128, N], U16) as ot3,
        nc.psum_tensor([128, HALF], F32) as psA,
        nc.psum_tensor([128, HALF], F32) as psB,
        nc.semaphore("k0_sem") as k0_sem,
        nc.semaphore("k1_sem") as k1_sem,
        nc.semaphore("k2_sem") as k2_sem,
        nc.semaphore("xq_sem") as xq_sem,
        nc.semaphore("pe_sem") as pe_sem,
        nc.semaphore("act_sem") as act_sem,
        nc.semaphore("od_sem") as od_sem,
        nc.Block() as block,
    ):
        stripes = [t1, t2, t3]
        ots = [ot0, ot1, ot2, ot3]
        pss = [psA, psB]

        def lhs(ki, m):  # stationary operand: x columns of stripe ki
            return stripes[ki][:, m * 128 : (m + 1) * 128]

        def rhs(ki, n):  # moving operand: s columns of stripe ki
            return stripes[ki][:, B_LOC + n * NB : B_LOC + (n + 1) * NB]

        @block.sync
        def _(sync):
            # ring A: first halves of the stripes (ring B takes the others)
            sync.dma_start(t1[:, 0:SPLIT], in1[:, 0:SPLIT]).then_inc(k0_sem, 16)
            sync.dma_start(t2[:, 0:SPLIT], in2[:, 0:SPLIT]).then_inc(k1_sem, 16)
            sync.dma_start(xq[:], xsq[:, :]).then_inc(xq_sem, 16)
            sync.dma_start(t3[:, 0:SPLIT], in3[:, 0:SPLIT]).then_inc(k2_sem, 16)
            for m in range(M_TILES):
                if m < M_TILES - 1:
                    sync.wait_ge(act_sem, 2 * (m + 1))
                    # full 128-row output stripe: contiguous 1 MB in DRAM
                    sync.dma_start(out[ts(m, 128), :], ots[m % 4][:]).then_inc(
                        od_sem, 16
                    )
                else:
                    # last stripe in halves so the final DMA tail is short
                    for h in range(2):
                        sync.wait_ge(act_sem, 2 * m + h + 1)
                        sync.dma_start(
                            out[ts(m, 128), ts(h, HALF)],
                            ots[m % 4][:, ts(h, HALF)],
                        ).then_inc(od_sem, 16)
            sync.wait_ge(od_sem, 16 * (M_TILES + 1))

        def emit_k(pe, hh, ki):
            m, h = hh // 2, hh % 2
            ps = pss[hh % 2]
            for nn in range(4):
                n = 4 * h + nn
                pe.matmul(
                    ps[:, ts(nn, NB)],
                    lhs(ki, m),
                    rhs(ki, n),
                    start=(ki == 0),
                    stop=False,
                )

        def emit_tail(pe, hh):
            # 4 concurrent K=32 matmuls in disjoint 32-row PE groups
            m, h = hh // 2, hh % 2
            ps = pss[hh % 2]
            for nn in range(4):
                n = 4 * h + nn
                mm = pe.matmul(
                    ps[:, ts(nn, NB)],
                    t3[ts(nn, 32), m * 128 : (m + 1) * 128],
                    t3[ts(nn, 32), B_LOC + n * NB : B_LOC + (n + 1) * NB],
                    start=False,
                    stop=True,
                    tile_position=(32 * nn, 0),
                )
                if nn == 3:
                    mm.then_inc(pe_sem, 1)

        @block.tensor
        def _(pe):
            # warm the HAM clock gate during the input load (psum garbage is
            # overwritten by the first start=True matmul of each half)
            for w in range(12):
                pe.matmul(psB[:, ts(w % 4, NB)], t1[:, 0:128],
                          t1[:, B_LOC : B_LOC + NB], start=True, stop=True)
            # first two halves interleaved with the staged input load
            pe.wait_ge(k0_sem, 32)
            emit_k(pe, 0, 0)
            emit_k(pe, 1, 0)
            pe.wait_ge(k1_sem, 32)
            emit_k(pe, 0, 1)
            pe.wait_ge(k2_sem, 32)  # tail stripe resident
            emit_tail(pe, 0)
            emit_k(pe, 1, 1)
            emit_tail(pe, 1)
            for hh in range(2, HALVES):
                # psum half reuse: ACT of half hh-2 must be done
                pe.wait_ge(act_sem, hh - 1)
                emit_k(pe, hh, 0)
                emit_k(pe, hh, 1)
                emit_tail(pe, hh)

        @block.scalar
        def _(act):
            # ring B: second halves of the stripes
            act.dma_start(t1[:, SPLIT:PACK], in1[:, SPLIT:PACK]).then_inc(
                k0_sem, 16
            )
            act.dma_start(t2[:, SPLIT:PACK], in2[:, SPLIT:PACK]).then_inc(
                k1_sem, 16
            )
            act.dma_start(t3[:, SPLIT:PACK], in3[:, SPLIT:PACK]).then_inc(
                k2_sem, 16
            )

            # dummy exp on scratch: hoists the ~2.7us ACT_TABLE_LOAD into the
            # input-load shadow instead of the first real eviction
            act.activation(scratch[:], scratch[:], mybir.ActivationFunctionType.Exp)
            act.wait_ge(xq_sem, 16)  # xq loaded
            for hh in range(HALVES):
                m, h = hh // 2, hh % 2
                if h == 0 and m >= 4:
                    # out row-tile reuse: DMA of row m-4 done
                    act.wait_ge(od_sem, 16 * (m - 3))
                act.wait_ge(pe_sem, hh + 1)
                act.activation(
                    ots[m % 4][:, ts(h, HALF)],
                    pss[hh % 2][:],
                    mybir.ActivationFunctionType.Exp,
                    bias=xq[:, m : m + 1],
                    scale=2.0 * GAMMA,
                ).then_inc(act_sem, 1)

    return nc


_NC_CACHE: bass.Bass | None = None


def _get_nc() -> bass.Bass:
    global _NC_CACHE
    if _NC_CACHE is None:
        _NC_CACHE = _build()
    return _NC_CACHE


def _prepare_in_maps(x: np.ndarray, s: np.ndarray) -> list[dict[str, np.ndarray]]:
    bf16 = ml_dtypes.bfloat16
    x = np.ascontiguousarray(np.asarray(x, dtype=np.float32))
    s = np.ascontiguousarray(np.asarray(s, dtype=np.float32))

    x64 = x.astype(np.float64)
    s64 = s.astype(np.float64)
    x_sq = np.einsum("bd,bd->b", x64, x64)
    s_sq = np.einsum("nd,nd->n", s64, s64)

    sT = s.T.astype(bf16)                    # (D, N)
    h = (-0.5 * s_sq).astype(np.float32)
    hi = h.astype(bf16)
    lo = (h - hi.astype(np.float32)).astype(bf16)
    tail = np.zeros((KTAIL, PACK), dtype=bf16)
    tail[0, 0:B_LOC] = 1
    tail[1, 0:B_LOC] = 1
    tail[0, B_LOC:] = hi
    tail[1, B_LOC:] = lo
    in3 = np.ascontiguousarray(np.tile(tail, (4, 1)))

    in_maps = []
    for c in range(NCORES):
        xc = x[c * B_LOC : (c + 1) * B_LOC]
        xTc = xc.T.astype(bf16)              # (D, B_LOC)
        in1 = np.concatenate([xTc[0:128], sT[0:128]], axis=1)
        in2 = np.concatenate([xTc[128:256], sT[128:256]], axis=1)
        xsq_c = np.ascontiguousarray(
            (np.log(OUT_SCALE) - GAMMA * x_sq[c * B_LOC : (c + 1) * B_LOC])
            .astype(np.float32)
            .reshape(M_TILES, 128)
            .T
        )
        in_maps.append(
            {
                "in1": np.ascontiguousarray(in1),
                "in2": np.ascontiguousarray(in2),
                "in3": in3,
                "xsq": xsq_c,
            }
        )
    return in_maps


def run(x: np.ndarray, s: np.ndarray, trace: bool = False, tmpdir: str | None = None):
    """Returns (full (8192, 4096) fp32 output, BassKernelResults)."""
    nc = _get_nc()
    in_maps = _prepare_in_maps(x, s)
    res = run_bass_kernel_spmd(
        nc, in_maps, core_ids=list(range(NCORES)), trace=trace, tmpdir=tmpdir
    )
    full = np.concatenate([np.asarray(r["out"]) for r in res.results], axis=0)
    full = full.astype(np.float32) * np.float32(1.0 / OUT_SCALE)
    return full, res


def kernel(**inputs: np.ndarray) -> np.ndarray:
    full, _ = run(inputs["inputs"], inputs["sample_matrix"], trace=False)
    return full


# revision 28
# speedup vs baseline: 1.0149x; 1.0149x over previous
"""RBF similarity: out[b, n] = exp(-gamma * ||inputs[b] - sample_matrix[n]||^2).

Strategy (8 trn2 NeuronCores, data-parallel over query rows):
  - Shard B=8192 query rows into 8 shards of 1024; replicate sample_matrix.
  - GEMM trick: -gamma*||x-s||^2 = 2g*x.s - g*||x||^2 - g*||s||^2.
  - Device computes psum = x_bf16.T @ s_bf16 over K=256, plus a K=32
    "tail" k-tile whose rows carry -0.5*||s||^2 (hi/lo bf16 split keeps
    the norm at ~fp32 precision). The tail weights live in 4
    partition-replicated 32-row groups so the 4 PSUM banks' K=32 matmuls
    run concurrently in disjoint row groups of the PE array
    (tile_position packing). Only rows 0/1 of each group are nonzero, so
    the tail s-rows are 4 tiny DMAs and the tail x-weights are memsets.
  - One ScalarE activation per 4-bank PSUM half evicts
    exp(2g*psum - g*||x||^2 + ln(65535)) as uint16 fixed point (the
    per-partition bias carries the per-row terms); the host decodes the
    uint16 transport back to fp32 in the gather (quantization error
    <= 0.5/65535, far below the bf16 matmul noise).
  - Raw bass with manual semaphores: the walrus build here allows at
    most one sync-wait per instruction, which Tile's scheduler exceeds.
  - Inputs arrive as 2 packed [128, 5120] bf16 stripes (x cols | s cols
    per k-group), each loaded by two DMAs on different HWDGE rings
    (sync + scalar) so per-engine descriptor latency overlaps; the PE
    starts right after stripe 1 and the first two halves interleave
    with the rest of the load.
"""

import numpy as np
import ml_dtypes

import concourse.bass as bass
import concourse.mybir as mybir
from concourse.bass import ts
from concourse.bass_utils import run_bass_kernel_spmd

GAMMA = 0.001
B, D, N = 8192, 256, 4096
NCORES = 8
B_LOC = B // NCORES          # 1024 query rows per core
M_TILES = B_LOC // 128       # 8 PSUM-partition tiles
KTAIL = 32                   # tail k-tile (rows 0/1: -0.5*||s||^2 hi/lo)
NB = 512                     # matmul free dim = one PSUM bank (fp32)
HALF = 2048                  # 4 banks per PSUM half
HALVES = 2 * M_TILES         # 16 half-iterations
PACK = B_LOC + N             # 5120: packed stripe width (x cols | s cols)
SPLIT = 2560                 # ring-split point of a stripe

BF16 = mybir.dt.bfloat16
F32 = mybir.dt.float32
U16 = mybir.dt.uint16
OUT_SCALE = 65535.0  # device writes round(out * 65535) as uint16; host rescales


def _build() -> bass.Bass:
    nc = bass.Bass(name="rbf_similarity", trn_type="TRN2")
    in1 = nc.dram_tensor("in1", [128, PACK], BF16, kind="ExternalInput")
    in2 = nc.dram_tensor("in2", [128, PACK], BF16, kind="ExternalInput")
    in3 = nc.dram_tensor("in3", [128, PACK], BF16, kind="ExternalInput")
    xsq = nc.dram_tensor("xsq", [128, M_TILES], F32, kind="ExternalInput")
    out = nc.dram_tensor("out", [B_LOC, N], U16, kind="ExternalOutput")

    with (
        nc.sbuf_tensor([128, PACK], BF16) as t1,
        nc.sbuf_tensor([128, PACK], BF16) as t2,
        nc.sbuf_tensor([128, PACK], BF16) as t3,
        nc.sbuf_tensor([128, M_TILES], F32) as xq,
        nc.sbuf_tensor([128, 1], F32) as scratch,
        nc.sbuf_tensor([128, 128 + NB], BF16) as wm,
        nc.sbuf_tensor([128, N], U16) as ot0,
        nc.sbuf_tensor([128, N], U16) as ot1,
        nc.sbuf_tensor([128, N], U16) as ot2,
        nc.sbuf_tensor([128, N], U16) as ot3,
        nc.psum_tensor([128, HALF], F32) as psA,
        nc.psum_tensor([128, HALF], F32) as psB,
        nc.semaphore("k0_sem") as k0_sem,
        nc.semaphore("k1_sem") as k1_sem,
        nc.semaphore("k2_sem") as k2_sem,
        nc.semaphore("xq_sem") as xq_sem,
        nc.semaphore("pe_sem") as pe_sem,
        nc.semaphore("act_sem") as act_sem,
        nc.semaphore("od_sem") as od_sem,
        nc.Block() as block,
    ):
        stripes = [t1, t2, t3]
        ots = [ot0, ot1, ot2, ot3]
        pss = [psA, psB]

        def lhs(ki, m):  # stationary operand: x columns of stripe ki
            return stripes[ki][:, m * 128 : (m + 1) * 128]

        def rhs(ki, n):  # moving operand: s columns of stripe ki
            return stripes[ki][:, B_LOC + n * NB : B_LOC + (n + 1) * NB]

        @block.sync
        def _(sync):
            # ring A: first halves of the stripes (ring B takes the others)
            sync.dma_start(t1[:, 0:SPLIT], in1[:, 0:SPLIT]).then_inc(k0_sem, 16)
            sync.dma_start(t2[:, 0:SPLIT], in2[:, 0:SPLIT]).then_inc(k1_sem, 16)
            sync.dma_start(xq[:], xsq[:, :]).then_inc(xq_sem, 16)
            sync.dma_start(t3[:, 0:SPLIT], in3[:, 0:SPLIT]).then_inc(k2_sem, 16)
            for m in range(M_TILES):
                if m < M_TILES - 1:
                    sync.wait_ge(act_sem, 2 * (m + 1))
                    # full 128-row output stripe: contiguous 1 MB in DRAM
                    sync.dma_start(out[ts(m, 128), :], ots[m % 4][:]).then_inc(
                        od_sem, 16
                    )
                else:
                    # last stripe in halves so the final DMA tail is short
                    for h in range(2):
                        sync.wait_ge(act_sem, 2 * m + h + 1)
                        sync.dma_start(
                            out[ts(m, 128), ts(h, HALF)],
                            ots[m % 4][:, ts(h, HALF)],
                        ).then_inc(od_sem, 16)
            sync.wait_ge(od_sem, 16 * (M_TILES + 1))

        def emit_k(pe, hh, ki):
            m, h = hh // 2, hh % 2
            ps = pss[hh % 2]
            for nn in range(4):
                n = 4 * h + nn
                pe.matmul(
                    ps[:, ts(nn, NB)],
                    lhs(ki, m),
                    rhs(ki, n),
                    start=(ki == 0),
                    stop=False,
                )

        def emit_tail(pe, hh):
            # 4 concurrent K=32 matmuls in disjoint 32-row PE groups
            m, h = hh // 2, hh % 2
            ps = pss[hh % 2]
            for nn in range(4):
                n = 4 * h + nn
                mm = pe.matmul(
                    ps[:, ts(nn, NB)],
                    t3[ts(nn, 32), m * 128 : (m + 1) * 128],
                    t3[ts(nn, 32), B_LOC + n * NB : B_LOC + (n + 1) * NB],
                    start=False,
                    stop=True,
                    tile_position=(32 * nn, 0),
                )
                if nn == 3:
                    mm.then_inc(pe_sem, 1)

        @block.tensor
        def _(pe):
            # warm the HAM clock gate during the input load (psum garbage is
            # overwritten by the first start=True matmul of each half)
            for w in range(12):
                pe.matmul(psB[:, ts(w % 4, NB)], wm[:, 0:128],
                          wm[:, 128 : 128 + NB], start=True, stop=True)
            # first two halves interleaved with the staged input load
            pe.wait_ge(k0_sem, 32)
            emit_k(pe, 0, 0)
            emit_k(pe, 1, 0)
            pe.wait_ge(k1_sem, 32)
            emit_k(pe, 0, 1)
            pe.wait_ge(k2_sem, 32)  # tail stripe resident
            emit_tail(pe, 0)
            emit_k(pe, 1, 1)
            emit_tail(pe, 1)
            for hh in range(2, HALVES):
                # psum half reuse: ACT of half hh-2 must be done
                pe.wait_ge(act_sem, hh - 1)
                emit_k(pe, hh, 0)
                emit_k(pe, hh, 1)
                emit_tail(pe, hh)

        @block.scalar
        def _(act):
            # ring B: second halves of the stripes
            act.dma_start(t1[:, SPLIT:PACK], in1[:, SPLIT:PACK]).then_inc(
                k0_sem, 16
            )
            act.dma_start(t2[:, SPLIT:PACK], in2[:, SPLIT:PACK]).then_inc(
                k1_sem, 16
            )
            act.dma_start(t3[:, SPLIT:PACK], in3[:, SPLIT:PACK]).then_inc(
                k2_sem, 16
            )

            # dummy exp on scratch: hoists the ~2.7us ACT_TABLE_LOAD into the
            # input-load shadow instead of the first real eviction
            act.activation(scratch[:], scratch[:], mybir.ActivationFunctionType.Exp)
            act.wait_ge(xq_sem, 16)  # xq loaded
            for hh in range(HALVES):
                m, h = hh // 2, hh % 2
                if h == 0 and m >= 4:
                    # out row-tile reuse: DMA of row m-4 done
                    act.wait_ge(od_sem, 16 * (m - 3))
                act.wait_ge(pe_sem, hh + 1)
                act.activation(
                    ots[m % 4][:, ts(h, HALF)],
                    pss[hh % 2][:],
                    mybir.ActivationFunctionType.Exp,
                    bias=xq[:, m : m + 1],
                    scale=2.0 * GAMMA,
                ).then_inc(act_sem, 1)

    return nc


_NC_CACHE: bass.Bass | None = None


def _get_nc() -> bass.Bass:
    global _NC_CACHE
    if _NC_CACHE is None:
        _NC_CACHE = _build()
    return _NC_CACHE


def _prepare_in_maps(x: np.ndarray, s: np.ndarray) -> list[dict[str, np.ndarray]]:
    bf16 = ml_dtypes.bfloat16
    x = np.ascontiguousarray(np.asarray(x, dtype=np.float32))
    s = np.ascontiguousarray(np.asarray(s, dtype=np.float32))

    x64 = x.astype(np.float64)
    s64 = s.astype(np.float64)
    x_sq = np.einsum("bd,bd->b", x64, x64)
    s_sq = np.einsum("nd,nd->n", s64, s64)

    sT = s.T.astype(bf16)                    # (D, N)
    h = (-0.5 * s_sq).astype(np.float32)
    hi = h.astype(bf16)
    lo = (h - hi.astype(np.float32)).astype(bf16)
    tail = np.zeros((KTAIL, PACK), dtype=bf16)
    tail[0, 0:B_LOC] = 1
    tail[1, 0:B_LOC] = 1
    tail[0, B_LOC:] = hi
    tail[1, B_LOC:] = lo
    in3 = np.ascontiguousarray(np.tile(tail, (4, 1)))

    in_maps = []
    for c in range(NCORES):
        xc = x[c * B_LOC : (c + 1) * B_LOC]
        xTc = xc.T.astype(bf16)              # (D, B_LOC)
        in1 = np.concatenate([xTc[0:128], sT[0:128]], axis=1)
        in2 = np.concatenate([xTc[128:256], sT[128:256]], axis=1)
        xsq_c = np.ascontiguousarray(
            (np.log(OUT_SCALE) - GAMMA * x_sq[c * B_LOC : (c + 1) * B_LOC])
            .astype(np.float32)
            .reshape(M_TILES, 128)
            .T
        )
        in_maps.append(
            {
                "in1": np.ascontiguousarray(in1),
                "in2": np.ascontiguousarray(in2),
                "in3": in3,
                "xsq": xsq_c,
            }
        )
    return in_maps


def run(x: np.ndarray, s: np.ndarray, trace: bool = False, tmpdir: str | None = None):
    """Returns (full (8192, 4096) fp32 output, BassKernelResults)."""
    nc = _get_nc()
    in_maps = _prepare_in_maps(x, s)
    res = run_bass_kernel_spmd(
        nc, in_maps, core_ids=list(range(NCORES)), trace=trace, tmpdir=tmpdir
    )
    full = np.concatenate([np.asarray(r["out"]) for r in res.results], axis=0)
    full = full.astype(np.float32) * np.float32(1.0 / OUT_SCALE)
    return full, res


def kernel(**inputs: np.ndarray) -> np.ndarray:
    full, _ = run(inputs["inputs"], inputs["sample_matrix"], trace=False)
    return full


# revision 29
# speedup vs baseline: 1.0370x; 1.0218x over previous
"""RBF similarity: out[b, n] = exp(-gamma * ||inputs[b] - sample_matrix[n]||^2).

Strategy (8 trn2 NeuronCores, data-parallel over query rows):
  - Shard B=8192 query rows into 8 shards of 1024; replicate sample_matrix.
  - GEMM trick: -gamma*||x-s||^2 = 2g*x.s - g*||x||^2 - g*||s||^2.
  - Device computes psum = x_bf16.T @ s_bf16 over K=256, plus a K=32
    "tail" k-tile whose rows carry -0.5*||s||^2 (hi/lo bf16 split keeps
    the norm at ~fp32 precision). The tail weights live in 4
    partition-replicated 32-row groups so the 4 PSUM banks' K=32 matmuls
    run concurrently in disjoint row groups of the PE array
    (tile_position packing). Only rows 0/1 of each group are nonzero, so
    the tail s-rows are 4 tiny DMAs and the tail x-weights are memsets.
  - One ScalarE activation per 4-bank PSUM half evicts
    exp(2g*psum - g*||x||^2 + ln(65535)) as uint16 fixed point (the
    per-partition bias carries the per-row terms); the host decodes the
    uint16 transport back to fp32 in the gather (quantization error
    <= 0.5/65535, far below the bf16 matmul noise).
  - Raw bass with manual semaphores: the walrus build here allows at
    most one sync-wait per instruction, which Tile's scheduler exceeds.
  - Inputs arrive as 2 packed [128, 5120] bf16 stripes (x cols | s cols
    per k-group), each loaded by two DMAs on different HWDGE rings
    (sync + scalar) so per-engine descriptor latency overlaps; the PE
    starts right after stripe 1 and the first two halves interleave
    with the rest of the load.
"""

import numpy as np
import ml_dtypes

import concourse.bass as bass
import concourse.mybir as mybir
from concourse.bass import ts
from concourse.bass_utils import run_bass_kernel_spmd

GAMMA = 0.001
B, D, N = 8192, 256, 4096
NCORES = 8
B_LOC = B // NCORES          # 1024 query rows per core
M_TILES = B_LOC // 128       # 8 PSUM-partition tiles
KTAIL = 32                   # tail k-tile (rows 0/1: -0.5*||s||^2 hi/lo)
NB = 512                     # matmul free dim = one PSUM bank (fp32)
HALF = 2048                  # 4 banks per PSUM half
HALVES = 2 * M_TILES         # 16 half-iterations
PACK = B_LOC + N             # 5120: packed stripe width (x cols | s cols)
SPLIT = 2560                 # ring-split point of a stripe

BF16 = mybir.dt.bfloat16
F32 = mybir.dt.float32
U16 = mybir.dt.uint16
OUT_SCALE = 65535.0  # device writes round(out * 65535) as uint16; host rescales


def _build() -> bass.Bass:
    nc = bass.Bass(name="rbf_similarity", trn_type="TRN2")
    in1 = nc.dram_tensor("in1", [128, PACK], BF16, kind="ExternalInput")
    in2 = nc.dram_tensor("in2", [128, PACK], BF16, kind="ExternalInput")
    in3 = nc.dram_tensor("in3", [128, PACK], BF16, kind="ExternalInput")
    xsq = nc.dram_tensor("xsq", [128, M_TILES], F32, kind="ExternalInput")
    out = nc.dram_tensor("out", [B_LOC, N], U16, kind="ExternalOutput")

    with (
        nc.sbuf_tensor([128, PACK], BF16) as t1,
        nc.sbuf_tensor([128, PACK], BF16) as t2,
        nc.sbuf_tensor([128, PACK], BF16) as t3,
        nc.sbuf_tensor([128, M_TILES], F32) as xq,
        nc.sbuf_tensor([128, 1], F32) as scratch,
        nc.sbuf_tensor([128, 128 + NB], BF16) as wm,
        nc.sbuf_tensor([128, N], U16) as ot0,
        nc.sbuf_tensor([128, N], U16) as ot1,
        nc.sbuf_tensor([128, N], U16) as ot2,
        nc.sbuf_tensor([128, N], U16) as ot3,
        nc.psum_tensor([128, HALF], F32) as psA,
        nc.psum_tensor([128, HALF], F32) as psB,
        nc.semaphore("k0_sem") as k0_sem,
        nc.semaphore("k1_sem") as k1_sem,
        nc.semaphore("k2_sem") as k2_sem,
        nc.semaphore("xq_sem") as xq_sem,
        nc.semaphore("pe_sem") as pe_sem,
        nc.semaphore("act_sem") as act_sem,
        nc.semaphore("od_sem") as od_sem,
        nc.Block() as block,
    ):
        stripes = [t1, t2, t3]
        ots = [ot0, ot1, ot2, ot3]
        pss = [psA, psB]

        def lhs(ki, m):  # stationary operand: x columns of stripe ki
            return stripes[ki][:, m * 128 : (m + 1) * 128]

        def rhs(ki, n):  # moving operand: s columns of stripe ki
            return stripes[ki][:, B_LOC + n * NB : B_LOC + (n + 1) * NB]

        @block.sync
        def _(sync):
            # ring A: first halves of the stripes (ring B takes the others)
            sync.dma_start(t1[:, 0:SPLIT], in1[:, 0:SPLIT]).then_inc(k0_sem, 16)
            sync.dma_start(t2[:, 0:SPLIT], in2[:, 0:SPLIT]).then_inc(k1_sem, 16)
            sync.dma_start(xq[:], xsq[:, :]).then_inc(xq_sem, 16)
            sync.dma_start(t3[:, 0:SPLIT], in3[:, 0:SPLIT]).then_inc(k2_sem, 16)
            for m in range(M_TILES):
                if m < M_TILES - 1:
                    sync.wait_ge(act_sem, 2 * (m + 1))
                    # full 128-row output stripe: contiguous 1 MB in DRAM
                    sync.dma_start(out[ts(m, 128), :], ots[m % 4][:]).then_inc(
                        od_sem, 16
                    )
                else:
                    # last stripe in halves so the final DMA tail is short
                    for h in range(2):
                        sync.wait_ge(act_sem, 2 * m + h + 1)
                        sync.dma_start(
                            out[ts(m, 128), ts(h, HALF)],
                            ots[m % 4][:, ts(h, HALF)],
                        ).then_inc(od_sem, 16)
            sync.wait_ge(od_sem, 16 * (M_TILES + 1))

        def emit_k(pe, hh, ki):
            m, h = hh // 2, hh % 2
            ps = pss[hh % 2]
            for nn in range(4):
                n = 4 * h + nn
                pe.matmul(
                    ps[:, ts(nn, NB)],
                    lhs(ki, m),
                    rhs(ki, n),
                    start=(ki == 0),
                    stop=False,
                )

        def emit_tail(pe, hh):
            # 4 concurrent K=32 matmuls in disjoint 32-row PE groups
            m, h = hh // 2, hh % 2
            ps = pss[hh % 2]
            for nn in range(4):
                n = 4 * h + nn
                mm = pe.matmul(
                    ps[:, ts(nn, NB)],
                    t3[ts(nn, 32), m * 128 : (m + 1) * 128],
                    t3[ts(nn, 32), B_LOC + n * NB : B_LOC + (n + 1) * NB],
                    start=False,
                    stop=True,
                    tile_position=(32 * nn, 0),
                )
                if nn == 3:
                    mm.then_inc(pe_sem, 1)

        @block.tensor
        def _(pe):
            # warm the HAM clock gate during the input load (psum garbage is
            # overwritten by the first start=True matmul of each half)
            for w in range(4):
                pe.matmul(psB[:, ts(w % 4, NB)], wm[:, 0:128],
                          wm[:, 128 : 128 + NB], start=True, stop=True)
            # first two halves interleaved with the staged input load
            pe.wait_ge(k0_sem, 32)
            emit_k(pe, 0, 0)
            emit_k(pe, 1, 0)
            pe.wait_ge(k1_sem, 32)
            emit_k(pe, 0, 1)
            pe.wait_ge(k2_sem, 32)  # tail stripe resident
            emit_tail(pe, 0)
            emit_k(pe, 1, 1)
            emit_tail(pe, 1)
            for hh in range(2, HALVES):
                # psum half reuse: ACT of half hh-2 must be done
                pe.wait_ge(act_sem, hh - 1)
                emit_k(pe, hh, 0)
                emit_k(pe, hh, 1)
                emit_tail(pe, hh)

        @block.scalar
        def _(act):
            # ring B: second halves of the stripes
            act.dma_start(t1[:, SPLIT:PACK], in1[:, SPLIT:PACK]).then_inc(
                k0_sem, 16
            )
            act.dma_start(t2[:, SPLIT:PACK], in2[:, SPLIT:PACK]).then_inc(
                k1_sem, 16
            )
            act.dma_start(t3[:, SPLIT:PACK], in3[:, SPLIT:PACK]).then_inc(
                k2_sem, 16
            )

            # dummy exp on scratch: hoists the ~2.7us ACT_TABLE_LOAD into the
            # input-load shadow instead of the first real eviction
            act.activation(scratch[:], scratch[:], mybir.ActivationFunctionType.Exp)
            act.wait_ge(xq_sem, 16)  # xq loaded
            for hh in range(HALVES):
                m, h = hh // 2, hh % 2
                if h == 0 and m >= 4:
                    # out row-tile reuse: DMA of row m-4 done
                    act.wait_ge(od_sem, 16 * (m - 3))
                act.wait_ge(pe_sem, hh + 1)
                act.activation(
                    ots[m % 4][:, ts(h, HALF)],
                    pss[hh % 2][:],
                    mybir.ActivationFunctionType.Exp,
                    bias=xq[:, m : m + 1],
                    scale=2.0 * GAMMA,
                ).then_inc(act_sem, 1)

    return nc


_NC_CACHE: bass.Bass | None = None


def _get_nc() -> bass.Bass:
    global _NC_CACHE
    if _NC_CACHE is None:
        _NC_CACHE = _build()
    return _NC_CACHE


def _prepare_in_maps(x: np.ndarray, s: np.ndarray) -> list[dict[str, np.ndarray]]:
    bf16 = ml_dtypes.bfloat16
    x = np.ascontiguousarray(np.asarray(x, dtype=np.float32))
    s = np.ascontiguousarray(np.asarray(s, dtype=np.float32))

    x64 = x.astype(np.float64)
    s64 = s.astype(np.float64)
    x_sq = np.einsum("bd,bd->b", x64, x64)
    s_sq = np.einsum("nd,nd->n", s64, s64)

    sT = s.T.astype(bf16)                    # (D, N)
    h = (-0.5 * s_sq).astype(np.float32)
    hi = h.astype(bf16)
    lo = (h - hi.astype(np.float32)).astype(bf16)
    tail = np.zeros((KTAIL, PACK), dtype=bf16)
    tail[0, 0:B_LOC] = 1
    tail[1, 0:B_LOC] = 1
    tail[0, B_LOC:] = hi
    tail[1, B_LOC:] = lo
    in3 = np.ascontiguousarray(np.tile(tail, (4, 1)))

    in_maps = []
    for c in range(NCORES):
        xc = x[c * B_LOC : (c + 1) * B_LOC]
        xTc = xc.T.astype(bf16)              # (D, B_LOC)
        in1 = np.concatenate([xTc[0:128], sT[0:128]], axis=1)
        in2 = np.concatenate([xTc[128:256], sT[128:256]], axis=1)
        xsq_c = np.ascontiguousarray(
            (np.log(OUT_SCALE) - GAMMA * x_sq[c * B_LOC : (c + 1) * B_LOC])
            .astype(np.float32)
            .reshape(M_TILES, 128)
            .T
        )
        in_maps.append(
            {
                "in1": np.ascontiguousarray(in1),
                "in2": np.ascontiguousarray(in2),
                "in3": in3,
                "xsq": xsq_c,
            }
        )
    return in_maps


def run(x: np.ndarray, s: np.ndarray, trace: bool = False, tmpdir: str | None = None):
    """Returns (full (8192, 4096) fp32 output, BassKernelResults)."""
    nc = _get_nc()
    in_maps = _prepare_in_maps(x, s)
    res = run_bass_kernel_spmd(
        nc, in_maps, core_ids=list(range(NCORES)), trace=trace, tmpdir=tmpdir
    )
    full = np.concatenate([np.asarray(r["out"]) for r in res.results], axis=0)
    full = full.astype(np.float32) * np.float32(1.0 / OUT_SCALE)
    return full, res


def kernel(**inputs: np.ndarray) -> np.ndarray:
    full, _ = run(inputs["inputs"], inputs["sample_matrix"], trace=False)
    return full
